# revision 9
# baseline (speedup 1.0000x reference)
"""Trainium2 Bass kernel: 4096x4096 fp32 'valid' cross-correlation with a 15x15
kernel, plus scalar bias.

Strategy
--------
- Shard output columns (W) across 8 NeuronCores: each core computes all 4082
  output rows for a 511-column stripe (core 7's tail columns are trimmed on
  the host). Each core's input stripe is its 511 columns plus a 14-column
  halo, gathered on the host -- inputs overlap, so no device-to-device
  communication is needed.
- On each core the 2D conv is computed as a sum of 15 banded-Toeplitz matmuls
  accumulated in PSUM: for each kernel column dj, a [K, M] Toeplitz matrix
  T_dj (T_dj[k, m] = weight[k-m, dj]) contracts up to 128 input rows against
  up to 114 output rows; the W-shift for dj is absorbed as a free-dim offset
  in the moving operand (image rows live in SBUF partitions, W along the free
  dim). All Toeplitz matrices are built on the host from the runtime weight.
- Inputs and Toeplitz weights are bf16 (measured 3.1e-3 max rel err vs the
  2e-2 tolerance): same-or-better matmul throughput than float32r at half
  the DMA traffic, and the 128-column padded stationary enables the
  compiler's fast weight load (FWL) path.
- Output is written as bf16 (halves output DMA; host upcasts to f32) on the
  Activation engine's DMA queue, so input loads on the SP queue are not
  serialized behind output stores -- chunk i+1's input DMA overlaps chunk
  i's matmuls.
"""

import numpy as np

H, W = 4096, 4096
KH, KW = 15, 15
HO, WO = H - KH + 1, W - KW + 1  # 4082, 4082
NCORES = 8
C = 512              # output cols per core (8*512 >= 4082)
CIN = C + KW - 1     # input cols per core stripe (with halo) = 526
MCH = 114            # output rows per h-chunk (114 + 14 = 128 = contraction K)

# h-chunks: (m0, Mc, K) -- Mc output rows from K = Mc+14 input rows
H_CHUNKS = [(b * MCH, min(MCH, HO - b * MCH), min(MCH, HO - b * MCH) + KH - 1)
            for b in range((HO + MCH - 1) // MCH)]
assert H_CHUNKS[-1][0] + H_CHUNKS[-1][2] == H  # last window ends exactly at H

_CACHE = {}


def _enable_ldw_opt():
    """Flip walrus --enable-ldw-opt to true (dedupes identical consecutive
    weight loads, which the dj-outer schedule produces)."""
    import concourse.bass_utils as bu
    if getattr(bu.run_command, "_ldw_patched", False):
        return
    orig = bu.run_command

    def patched(argv, **kw):
        argv = ["--enable-ldw-opt=true" if a == "--enable-ldw-opt=false" else a
                for a in argv]
        return orig(argv, **kw)

    patched._ldw_patched = True
    bu.run_command = patched


def _build_nc(reps: int = 1, scheme: str = "dji", kw_used: int = KW,
              xbufs: int = 3, ldw_opt: bool = False, grp: int = 4,
              out_q: str = "act", out_bf16: bool = True, uk: bool = False):
    import concourse.bacc as bacc
    import concourse.mybir as mybir
    from concourse.tile import TileContext

    if ldw_opt:
        _enable_ldw_opt()

    f32 = mybir.dt.float32
    bf16 = mybir.dt.bfloat16

    nc = bacc.Bacc("TRN2", debug=False, num_devices=NCORES)
    xs_d = nc.dram_tensor("xs", [H, CIN], bf16, kind="ExternalInput")
    wT_d = nc.dram_tensor("wT", [128, KW, 128], bf16, kind="ExternalInput")
    bias_d = nc.dram_tensor("bias", [1, 1], f32, kind="ExternalInput")
    out_dt = bf16 if out_bf16 else f32
    ys_d = nc.dram_tensor("ys", [HO, C], out_dt, kind="ExternalOutput")

    out_eng = {"sp": nc.sync, "act": nc.scalar}[out_q]

    # uk: uniform K=128 chunks; the last chunk starts earlier (overlapping
    # the previous one) and its first `skip` psum rows are not written out.
    if uk:
        chunks = [(m0, 0, Mc, K) for (m0, Mc, K) in H_CHUNKS[:-1]]
        m0_last = HO - MCH  # 3968
        skip = H_CHUNKS[-1][0] - m0_last  # rows already written by prev chunk
        chunks.append((m0_last, skip, MCH, 128))
    else:
        chunks = [(m0, 0, Mc, K) for (m0, Mc, K) in H_CHUNKS]

    with TileContext(nc) as tc:
        with (
            tc.tile_pool(name="xp", bufs=xbufs if scheme == "dji" else 2 * grp) as xp,
            tc.tile_pool(name="wp", bufs=1) as wp,
            tc.tile_pool(name="op", bufs=6) as op,
            tc.tile_pool(name="pp", bufs=4 if scheme == "dji" else 2 * grp,
                         space="PSUM") as pp,
        ):
            # Weights (Toeplitz stack) + bias
            w_t = wp.tile([128, KW, 128], bf16)
            nc.sync.dma_start(w_t[:, :, :], wT_d[:, :, :])
            bias_t = wp.tile([1, 1], f32)
            nc.sync.dma_start(bias_t[:, :], bias_d[:, :])
            bias_bc = wp.tile([128, 1], f32)
            nc.gpsimd.partition_broadcast(bias_bc[:, :], bias_t[:, :])

            for _rep in range(reps):
                if scheme == "dji":
                    for m0, sk, Mc, K in chunks:
                        x_b = xp.tile([128, CIN], bf16, name="x_b")
                        nc.sync.dma_start(x_b[0:K, :], xs_d[m0:m0 + K, :])
                        ps = pp.tile([128, C], f32, name="ps")
                        for dj in range(kw_used):
                            nc.tensor.matmul(
                                ps[0:128, 0:C],
                                w_t[0:K, dj, 0:128],
                                x_b[0:K, dj:dj + C],
                                start=(dj == 0),
                                stop=(dj == kw_used - 1),
                            )
                        o = op.tile([MCH, C], out_dt, name="o")
                        nc.vector.tensor_scalar_add(
                            o[0:Mc - sk, 0:C], ps[sk:Mc, 0:C],
                            bias_bc[0:Mc - sk, 0:1]
                        )
                        out_eng.dma_start(ys_d[m0 + sk:m0 + Mc, 0:C],
                                          o[0:Mc - sk, 0:C])
                else:  # "djo": dj-outer over groups of `grp` chunks
                    for g0 in range(0, len(chunks), grp):
                        grp_chunks = chunks[g0:g0 + grp]
                        xts = []
                        for i, (m0, sk, Mc, K) in enumerate(grp_chunks):
                            x_b = xp.tile([128, CIN], bf16, name=f"x{i}")
                            nc.sync.dma_start(x_b[0:K, :], xs_d[m0:m0 + K, :])
                            xts.append(x_b)
                        pss = [pp.tile([128, C], f32, name=f"ps{i}")
                               for i in range(len(grp_chunks))]
                        for dj in range(kw_used):
                            for i, (m0, sk, Mc, K) in enumerate(grp_chunks):
                                nc.tensor.matmul(
                                    pss[i][0:128, 0:C],
                                    w_t[0:K, dj, 0:128],
                                    xts[i][0:K, dj:dj + C],
                                    start=(dj == 0),
                                    stop=(dj == kw_used - 1),
                                    skip_group_check=True,
                                )
                        for i, (m0, sk, Mc, K) in enumerate(grp_chunks):
                            o = op.tile([MCH, C], out_dt, name="o")
                            nc.vector.tensor_scalar_add(
                                o[0:Mc - sk, 0:C], pss[i][sk:Mc, 0:C],
                                bias_bc[0:Mc - sk, 0:1]
                            )
                            out_eng.dma_start(
                                ys_d[m0 + sk:m0 + Mc, 0:C], o[0:Mc - sk, 0:C]
                            )

    nc.compile()
    return nc


def _toeplitz_stack(weight: np.ndarray) -> np.ndarray:
    """wT[k, dj, m] = weight[k-m, dj] for 0 <= k-m < KH (bf16, M padded to
    128; columns m >= 114 produce garbage psum rows that are never read)."""
    import ml_dtypes
    wT = np.zeros((128, KW, 128), dtype=np.float32)
    for di in range(KH):
        for m in range(128):
            if m + di < 128:
                wT[m + di, :, m] = weight[di, :]
    return wT.astype(ml_dtypes.bfloat16)


def _build_in_maps(x, weight, bias):
    import ml_dtypes
    x = np.ascontiguousarray(x, dtype=np.float32)
    weight = np.asarray(weight, dtype=np.float32)
    bias_v = np.asarray(bias, dtype=np.float32).reshape(-1)[:1]

    x_pad = np.zeros((H, NCORES * C + KW - 1), dtype=np.float32)
    x_pad[:, :W] = x
    x_pad = x_pad.astype(ml_dtypes.bfloat16)
    wT = _toeplitz_stack(weight)
    bias_in = bias_v.reshape(1, 1)

    return [
        {"xs": np.ascontiguousarray(x_pad[:, c * C:c * C + CIN]),
         "wT": wT, "bias": bias_in}
        for c in range(NCORES)
    ]


def kernel(x: np.ndarray, weight: np.ndarray, bias: np.ndarray) -> np.ndarray:
    from concourse.bass_utils import run_bass_kernel_spmd

    if "nc" not in _CACHE:
        _CACHE["nc"] = _build_nc()
    nc = _CACHE["nc"]

    in_maps = _build_in_maps(x, weight, bias)
    res = run_bass_kernel_spmd(nc, in_maps, core_ids=list(range(NCORES)))

    out = np.empty((HO, WO), dtype=np.float32)
    for c in range(NCORES):
        c0 = c * C
        c1 = min(c0 + C, WO)
        out[:, c0:c1] = res.results[c]["ys"][:, : c1 - c0].astype(np.float32)
    return out


# revision 10
# speedup vs baseline: 1.4032x; 1.4032x over previous
"""Trainium2 Bass kernel: 4096x4096 fp32 'valid' cross-correlation with a 15x15
kernel, plus scalar bias.

Strategy
--------
- Shard output columns (W) across 8 NeuronCores: each core computes all 4082
  output rows for a 511-column stripe (core 7's tail columns are trimmed on
  the host). Each core's input stripe is its 511 columns plus a 14-column
  halo, gathered on the host -- inputs overlap, so no device-to-device
  communication is needed.
- On each core the 2D conv is computed as a sum of 15 banded-Toeplitz matmuls
  accumulated in PSUM: for each kernel column dj, a [K, M] Toeplitz matrix
  T_dj (T_dj[k, m] = weight[k-m, dj]) contracts up to 128 input rows against
  up to 114 output rows; the W-shift for dj is absorbed as a free-dim offset
  in the moving operand (image rows live in SBUF partitions, W along the free
  dim). All Toeplitz matrices are built on the host from the runtime weight.
- Inputs and Toeplitz weights are bf16 (measured 3.1e-3 max rel err vs the
  2e-2 tolerance): same-or-better matmul throughput than float32r at half
  the DMA traffic, and the 128-column padded stationary enables the
  compiler's fast weight load (FWL) path.
- Output is written as bf16 (halves output DMA; host upcasts to f32) on the
  Activation engine's DMA queue, so input loads on the SP queue are not
  serialized behind output stores -- chunk i+1's input DMA overlaps chunk
  i's matmuls.
"""

import numpy as np

H, W = 4096, 4096
KH, KW = 15, 15
HO, WO = H - KH + 1, W - KW + 1  # 4082, 4082
NCORES = 8
C = 512              # output cols per core (8*512 >= 4082)
CIN = C + KW - 1     # input cols per core stripe (with halo) = 526
MCH = 114            # output rows per h-chunk (114 + 14 = 128 = contraction K)

# h-chunks: (m0, Mc, K) -- Mc output rows from K = Mc+14 input rows
H_CHUNKS = [(b * MCH, min(MCH, HO - b * MCH), min(MCH, HO - b * MCH) + KH - 1)
            for b in range((HO + MCH - 1) // MCH)]
assert H_CHUNKS[-1][0] + H_CHUNKS[-1][2] == H  # last window ends exactly at H

_CACHE = {}


def _enable_ldw_opt():
    """Flip walrus --enable-ldw-opt to true (dedupes identical consecutive
    weight loads, which the dj-outer schedule produces)."""
    import concourse.bass_utils as bu
    if getattr(bu.run_command, "_ldw_patched", False):
        return
    orig = bu.run_command

    def patched(argv, **kw):
        argv = ["--enable-ldw-opt=true" if a == "--enable-ldw-opt=false" else a
                for a in argv]
        return orig(argv, **kw)

    patched._ldw_patched = True
    bu.run_command = patched


def _build_nc(reps: int = 1, scheme: str = "djo", kw_used: int = KW,
              xbufs: int = 3, ldw_opt: bool = False, grp: int = 2,
              out_q: str = "act", out_bf16: bool = True, uk: bool = False):
    import concourse.bacc as bacc
    import concourse.mybir as mybir
    from concourse.tile import TileContext

    if ldw_opt:
        _enable_ldw_opt()

    f32 = mybir.dt.float32
    bf16 = mybir.dt.bfloat16

    nc = bacc.Bacc("TRN2", debug=False, num_devices=NCORES)
    xs_d = nc.dram_tensor("xs", [H, CIN], bf16, kind="ExternalInput")
    wT_d = nc.dram_tensor("wT", [128, KW, 128], bf16, kind="ExternalInput")
    bias_d = nc.dram_tensor("bias", [1, 1], f32, kind="ExternalInput")
    out_dt = bf16 if out_bf16 else f32
    ys_d = nc.dram_tensor("ys", [HO, C], out_dt, kind="ExternalOutput")

    out_eng = {"sp": nc.sync, "act": nc.scalar}[out_q]

    # uk: uniform K=128 chunks; the last chunk starts earlier (overlapping
    # the previous one) and its first `skip` psum rows are not written out.
    if uk:
        chunks = [(m0, 0, Mc, K) for (m0, Mc, K) in H_CHUNKS[:-1]]
        m0_last = HO - MCH  # 3968
        skip = H_CHUNKS[-1][0] - m0_last  # rows already written by prev chunk
        chunks.append((m0_last, skip, MCH, 128))
    else:
        chunks = [(m0, 0, Mc, K) for (m0, Mc, K) in H_CHUNKS]

    with TileContext(nc) as tc:
        with (
            tc.tile_pool(name="xp", bufs=xbufs if scheme == "dji" else 2 * grp) as xp,
            tc.tile_pool(name="wp", bufs=1) as wp,
            tc.tile_pool(name="op", bufs=6) as op,
            tc.tile_pool(name="pp", bufs=4 if scheme == "dji" else 2 * grp,
                         space="PSUM") as pp,
        ):
            # Weights (Toeplitz stack) + bias
            w_t = wp.tile([128, KW, 128], bf16)
            nc.sync.dma_start(w_t[:, :, :], wT_d[:, :, :])
            bias_t = wp.tile([1, 1], f32)
            nc.sync.dma_start(bias_t[:, :], bias_d[:, :])
            bias_bc = wp.tile([128, 1], f32)
            nc.gpsimd.partition_broadcast(bias_bc[:, :], bias_t[:, :])

            for _rep in range(reps):
                if scheme == "dji":
                    for m0, sk, Mc, K in chunks:
                        x_b = xp.tile([128, CIN], bf16, name="x_b")
                        nc.sync.dma_start(x_b[0:K, :], xs_d[m0:m0 + K, :])
                        ps = pp.tile([128, C], f32, name="ps")
                        for dj in range(kw_used):
                            nc.tensor.matmul(
                                ps[0:128, 0:C],
                                w_t[0:K, dj, 0:128],
                                x_b[0:K, dj:dj + C],
                                start=(dj == 0),
                                stop=(dj == kw_used - 1),
                            )
                        o = op.tile([MCH, C], out_dt, name="o")
                        nc.vector.tensor_scalar_add(
                            o[0:Mc - sk, 0:C], ps[sk:Mc, 0:C],
                            bias_bc[0:Mc - sk, 0:1]
                        )
                        out_eng.dma_start(ys_d[m0 + sk:m0 + Mc, 0:C],
                                          o[0:Mc - sk, 0:C])
                else:  # "djo": dj-outer over groups of `grp` chunks
                    for g0 in range(0, len(chunks), grp):
                        grp_chunks = chunks[g0:g0 + grp]
                        xts = []
                        for i, (m0, sk, Mc, K) in enumerate(grp_chunks):
                            x_b = xp.tile([128, CIN], bf16, name=f"x{i}")
                            nc.sync.dma_start(x_b[0:K, :], xs_d[m0:m0 + K, :])
                            xts.append(x_b)
                        pss = [pp.tile([128, C], f32, name=f"ps{i}")
                               for i in range(len(grp_chunks))]
                        for dj in range(kw_used):
                            for i, (m0, sk, Mc, K) in enumerate(grp_chunks):
                                nc.tensor.matmul(
                                    pss[i][0:128, 0:C],
                                    w_t[0:K, dj, 0:128],
                                    xts[i][0:K, dj:dj + C],
                                    start=(dj == 0),
                                    stop=(dj == kw_used - 1),
                                    skip_group_check=True,
                                )
                        for i, (m0, sk, Mc, K) in enumerate(grp_chunks):
                            o = op.tile([MCH, C], out_dt, name="o")
                            nc.vector.tensor_scalar_add(
                                o[0:Mc - sk, 0:C], pss[i][sk:Mc, 0:C],
                                bias_bc[0:Mc - sk, 0:1]
                            )
                            out_eng.dma_start(
                                ys_d[m0 + sk:m0 + Mc, 0:C], o[0:Mc - sk, 0:C]
                            )

    nc.compile()
    return nc


def _toeplitz_stack(weight: np.ndarray) -> np.ndarray:
    """wT[k, dj, m] = weight[k-m, dj] for 0 <= k-m < KH (bf16, M padded to
    128; columns m >= 114 produce garbage psum rows that are never read)."""
    import ml_dtypes
    wT = np.zeros((128, KW, 128), dtype=np.float32)
    for di in range(KH):
        for m in range(128):
            if m + di < 128:
                wT[m + di, :, m] = weight[di, :]
    return wT.astype(ml_dtypes.bfloat16)


def _build_in_maps(x, weight, bias):
    import ml_dtypes
    x = np.ascontiguousarray(x, dtype=np.float32)
    weight = np.asarray(weight, dtype=np.float32)
    bias_v = np.asarray(bias, dtype=np.float32).reshape(-1)[:1]

    x_pad = np.zeros((H, NCORES * C + KW - 1), dtype=np.float32)
    x_pad[:, :W] = x
    x_pad = x_pad.astype(ml_dtypes.bfloat16)
    wT = _toeplitz_stack(weight)
    bias_in = bias_v.reshape(1, 1)

    return [
        {"xs": np.ascontiguousarray(x_pad[:, c * C:c * C + CIN]),
         "wT": wT, "bias": bias_in}
        for c in range(NCORES)
    ]


def kernel(x: np.ndarray, weight: np.ndarray, bias: np.ndarray) -> np.ndarray:
    from concourse.bass_utils import run_bass_kernel_spmd

    if "nc" not in _CACHE:
        _CACHE["nc"] = _build_nc()
    nc = _CACHE["nc"]

    in_maps = _build_in_maps(x, weight, bias)
    res = run_bass_kernel_spmd(nc, in_maps, core_ids=list(range(NCORES)))

    out = np.empty((HO, WO), dtype=np.float32)
    for c in range(NCORES):
        c0 = c * C
        c1 = min(c0 + C, WO)
        out[:, c0:c1] = res.results[c]["ys"][:, : c1 - c0].astype(np.float32)
    return out
